# revision 31
# baseline (speedup 1.0000x reference)
"""BitLinear (RMSNorm + int8 act quant + ternary weight GEMM) on 8 TRN2 cores.

Weights are static, so the ternary quantization (w_scale = mean|w|, w_q =
clip(round(w/w_scale))) is precomputed on host; w_q ships as fp8e4 ({-1,0,1}
is exact) and w_scale is folded into the output on host. The device runs
only the per-token pipeline: no weight passes, no collectives.

Sharding: 2 token-groups x 4 dout-groups. Each core:
  - x shard [4096, 2048] f32 (token-parallel)
  - wq shard = ternary(weight)[og*2048:(og+1)*2048, :].T as fp8e4,
    host-tiled to [oc, p, k, c] so each oc chunk is one DMA with
    16KB-contiguous runs
  - gw: norm_weight replicated across 128 partitions
Device pipeline per core, per 128-token tile:
  sum(x^2) (ACT Square+accum), max|x*g| (DVE reduce), per-token scalars,
  q = round((x*g)*m) via magic-constant RNE round -> bf16,
  DMA-transpose q to [d, t] layout, 64 bf16xfp8 matmuls into PSUM
  (mixed-dtype runs at full bf16 rate, verified on HW),
  scale by x_scale on PSUM->SBUF copy (bf16), DMA out.
The quantized GEMM is exact: x_q in [-127,127] (bf16) and w_q in {-1,0,1}
(fp8) are exact, PSUM accumulates in f32. The only approximation vs the f32
reference is the bf16 rounding of the output (~2e-3 rel), far inside the
2e-2 gate.

Startup schedule (the three dynamic DMA queues share one DGE engine at
~130 GB/s per queue; descriptor count and byte order decide the time to
first matmul): tile 0's x and gw load as column halves on separate queues,
weight oc chunks are interleaved with the early x loads in need-order, and
the sync queue carries only transposes + outputs so tile 0's transpose is
never stuck behind bulk traffic. The out DMA for tile i is emitted after
tile i+1's transpose (in-order queue), and the last tile's output goes out
per oc chunk to shorten the tail.
"""

import sys

if "/opt/trn_rl_repo" not in sys.path:
    sys.path.insert(0, "/opt/trn_rl_repo")

import numpy as np
import ml_dtypes

# ---------------------------------------------------------------- config

N_CORES = 8
TG, OG = 2, 4            # token groups x dout groups
B, S, DIN, DOUT = 4, 2048, 2048, 8192
TOKENS = B * S           # 8192
T_SH = TOKENS // TG      # 4096 tokens per core
O_SH = DOUT // OG        # 2048 dout per core

P = 128                  # partitions
EPS_NORM = 1e-6
EPS_SCALE = 1e-8
QB = 127.0
C_MAGIC = 12582912.0     # 1.5 * 2^23 : float32 RNE integer-rounding constant


def build_bass(t_sh=T_SH, din=DIN, o_sh=O_SH, n_cores=N_CORES):
    """Build the per-core SPMD Bass graph. Shapes parametrized for sim tests."""
    import concourse.bass as bass
    import concourse.bacc as bacc
    import concourse.mybir as mybir
    from concourse import masks, tile

    fp32 = mybir.dt.float32
    bf16 = mybir.dt.bfloat16
    fp8 = mybir.dt.float8e4
    Alu = mybir.AluOpType
    Act = mybir.ActivationFunctionType

    t_tiles = t_sh // P          # token tiles
    k_tiles = din // P           # contraction tiles
    oc_sz = 512 if o_sh >= 512 else o_sh
    oc_chunks = o_sh // oc_sz    # PSUM output chunks per token tile
    h = din // 2

    nc = bacc.Bacc("TRN2", target_bir_lowering=False, debug=False,
                   num_devices=n_cores)

    x_d = nc.dram_tensor("x", [t_sh, din], fp32, kind="ExternalInput")
    wq_d = nc.dram_tensor("wq", [oc_chunks, P, k_tiles, oc_sz], fp8,
                          kind="ExternalInput")
    gw_d = nc.dram_tensor("gw", [P, din], fp32, kind="ExternalInput")
    out_d = nc.dram_tensor("out", [t_sh, o_sh], bf16, kind="ExternalOutput")

    with tile.TileContext(nc) as tc:
        with (
            tc.tile_pool(name="persist", bufs=1) as persist,
            tc.tile_pool(name="xin", bufs=2) as xin_pool,
            tc.tile_pool(name="ybuf", bufs=2) as y_pool,
            tc.tile_pool(name="t1buf", bufs=2) as t1_pool,
            tc.tile_pool(name="qbuf", bufs=2) as q_pool,
            tc.tile_pool(name="qtbuf", bufs=3) as qt_pool,
            tc.tile_pool(name="obuf", bufs=2) as out_pool,
            tc.tile_pool(name="small", bufs=4) as small,
            tc.tile_pool(name="psum", bufs=6, space="PSUM") as psum_pool,
        ):
            # ---------------- persistent tiles
            gw_sb = persist.tile([P, din], fp32)
            nc.scalar.dma_start(gw_sb[:, 0:h], gw_d[:, 0:h])
            nc.sync.dma_start(gw_sb[:, h:], gw_d[:, h:])
            epsn_sb = persist.tile([P, 1], fp32)
            nc.gpsimd.memset(epsn_sb[:], EPS_NORM)
            # identity for tile 0's PE-based transpose (the first xbar DMA
            # transpose is framework-gated until ~37us; the PE is idle then)
            ident_sb = persist.tile([P, P], bf16)
            masks.make_identity(nc, ident_sb[:])

            wqall = persist.tile([P, oc_chunks, k_tiles, oc_sz], fp8,
                                 name="wqall")

            # per-token stats, one column per token tile
            sumsq_t = persist.tile([P, t_tiles], fp32)
            amax_t = persist.tile([P, t_tiles], fp32)
            m_t = persist.tile([P, t_tiles], fp32)
            alpha_t = persist.tile([P, t_tiles], fp32)

            # ---------------- x loop (fully per-tile; avoids pool-slot cycles)
            osb_prev = None
            for i in range(t_tiles):
                    xt = xin_pool.tile([P, din], fp32, tag="xin")
                    yt = y_pool.tile([P, din], fp32, tag="y")
                    scr = t1_pool.tile([P, din], fp32, tag="t1")
                    if i == 0:
                        # tile 0 is the startup critical path: column halves,
                        # each half's x load on its own DMA queue. The weight
                        # chunk triggers are emitted here, after the x0
                        # halves, so they sit behind x0 in queue order.
                        ha, hb = slice(0, h), slice(h, din)
                        nc.scalar.dma_start(xt[:, ha], x_d[0:P, ha])
                        nc.gpsimd.dma_start(xt[:, hb], x_d[0:P, hb])
                        # weight chunks spread over all three queues; sync is
                        # safe to use now that tile 0 transposes on the PE
                        # (tile 1's DMA transpose isn't needed until well
                        # after oc2 drains). gpsimd keeps oc0+oc3 only, so
                        # every chunk lands before its matmuls need it even
                        # with the PE starting at ~28us.
                        wq_eng = {1: nc.scalar, 2: nc.sync}
                        for oc in range(oc_chunks):
                            wq_eng.get(oc, nc.gpsimd).dma_start(
                                wqall[:, oc, :, :], wq_d[oc])
                        ssa = small.tile([P, 1], fp32, tag="ssa")
                        ama = small.tile([P, 1], fp32, tag="ama")
                        ssb = small.tile([P, 1], fp32, tag="ssb")
                        amb = small.tile([P, 1], fp32, tag="amb")
                        for hs, (sst, amt) in ((ha, (ssa, ama)),
                                               (hb, (ssb, amb))):
                            nc.vector.tensor_tensor(out=yt[:, hs],
                                                    in0=xt[:, hs],
                                                    in1=gw_sb[:, hs],
                                                    op=Alu.mult)
                            nc.scalar.activation(scr[:, hs], xt[:, hs],
                                                 Act.Square, accum_out=sst[:])
                            nc.vector.tensor_reduce(
                                out=amt[:], in_=yt[:, hs], op=Alu.max,
                                axis=mybir.AxisListType.X,
                                apply_absolute_value=True)
                        nc.vector.tensor_tensor(out=sumsq_t[:, 0:1],
                                                in0=ssa[:], in1=ssb[:],
                                                op=Alu.add)
                        nc.vector.tensor_tensor(out=amax_t[:, 0:1],
                                                in0=ama[:], in1=amb[:],
                                                op=Alu.max)
                    else:
                        nc.scalar.dma_start(xt[:], x_d[i * P:(i + 1) * P, :])
                        nc.vector.tensor_tensor(out=yt[:], in0=xt[:],
                                                in1=gw_sb[:], op=Alu.mult)
                        nc.scalar.activation(scr[:], xt[:], Act.Square,
                                             accum_out=sumsq_t[:, i:i + 1])
                        nc.vector.tensor_reduce(out=amax_t[:, i:i + 1],
                                                in_=yt[:], op=Alu.max,
                                                axis=mybir.AxisListType.X,
                                                apply_absolute_value=True)
                    # per-token scalars on [P, 1]
                    # sq = sqrt(sumsq/din + eps_norm)  (scale+bias fused)
                    sq = small.tile([P, 1], fp32, tag="sq")
                    nc.scalar.activation(sq[:], sumsq_t[:, i:i + 1], Act.Sqrt,
                                         scale=1.0 / din, bias=epsn_sb[:])
                    d1 = small.tile([P, 1], fp32, tag="d1")
                    nc.vector.tensor_scalar(out=d1[:], in0=amax_t[:, i:i + 1],
                                            scalar1=1.0 / QB, scalar2=None,
                                            op0=Alu.mult)
                    # f1 = d1 + EPS_SCALE*sq ; m = 1/f1
                    e1 = small.tile([P, 1], fp32, tag="e1")
                    nc.vector.tensor_scalar(out=e1[:], in0=sq[:], scalar1=EPS_SCALE,
                                            scalar2=None, op0=Alu.mult)
                    f1 = small.tile([P, 1], fp32, tag="f1")
                    nc.vector.tensor_tensor(out=f1[:], in0=d1[:], in1=e1[:],
                                            op=Alu.add)
                    nc.vector.reciprocal(m_t[:, i:i + 1], f1[:])
                    rsq = small.tile([P, 1], fp32, tag="rsq")
                    nc.vector.reciprocal(rsq[:], sq[:])
                    xs0 = small.tile([P, 1], fp32, tag="xs0")
                    nc.vector.tensor_tensor(out=xs0[:], in0=d1[:], in1=rsq[:],
                                            op=Alu.mult)
                    # alpha = xs0 + eps  (w_scale is applied on host)
                    nc.vector.tensor_scalar(out=alpha_t[:, i:i + 1], in0=xs0[:],
                                            scalar1=EPS_SCALE, scalar2=None,
                                            op0=Alu.add)
                    # quantize; one xbar transpose per chunk: out[d_lo, k, t]
                    # = qt8[t, 128k + d_lo]  (verified blocked layout on HW)
                    t1 = t1_pool.tile([P, din], fp32, tag="t1")
                    qt8 = q_pool.tile([P, din], bf16, tag="q")
                    qT = qt_pool.tile([P, k_tiles, P], bf16, tag="qT")
                    halves = ((slice(0, h), slice(0, k_tiles // 2)),
                              (slice(h, din), slice(k_tiles // 2, k_tiles))) \
                        if i == 0 else ((slice(0, din), slice(0, k_tiles)),)
                    for hs, kg in halves:
                        nc.vector.tensor_scalar(out=t1[:, hs], in0=yt[:, hs],
                                                scalar1=m_t[:, i:i + 1],
                                                scalar2=C_MAGIC,
                                                op0=Alu.mult, op1=Alu.add)
                        nc.vector.tensor_scalar(out=qt8[:, hs], in0=t1[:, hs],
                                                scalar1=C_MAGIC,
                                                scalar2=None, op0=Alu.subtract)
                        if i == 0:
                            # PE transpose, per k block, dodging the gated
                            # first xbar transpose (the PE idles until the
                            # weights land anyway)
                            for k in range(kg.start, kg.stop):
                                ptt = psum_pool.tile([P, P], bf16, tag="tp",
                                                     bufs=2)
                                nc.tensor.matmul(ptt[:],
                                                 qt8[:, k * P:(k + 1) * P],
                                                 ident_sb[:],
                                                 is_transpose=True)
                                nc.vector.tensor_copy(qT[:, k, :], ptt[:])
                        else:
                            nc.sync.dma_start(out=qT[:, kg, :],
                                              in_=qt8[:, hs], transpose=True)
                    # previous tile's output, after this tile's transpose so
                    # the in-order sync stream can't stall the transpose
                    if osb_prev is not None:
                        nc.sync.dma_start(out_d[(i - 1) * P:i * P, :],
                                          osb_prev[:])
                    osb = out_pool.tile([P, o_sh], bf16, tag="o")
                    for oc in range(oc_chunks):
                        osl = slice(oc * oc_sz, (oc + 1) * oc_sz)
                        pt = psum_pool.tile([P, oc_sz], fp32, tag="ps")
                        for k in range(k_tiles):
                            nc.tensor.matmul(pt[:], qT[:, k, :],
                                             wqall[:, oc, k, :],
                                             start=(k == 0), stop=(k == k_tiles - 1))
                        nc.scalar.activation(osb[:, osl], pt[:], Act.Copy,
                                             scale=alpha_t[:, i:i + 1])
                        if i == t_tiles - 1:
                            # last tile: ship each oc chunk as soon as its
                            # PSUM drain lands, shortening the tail
                            nc.sync.dma_start(out_d[i * P:(i + 1) * P, osl],
                                              osb[:, osl])
                    osb_prev = osb

    nc.compile()
    return nc


# ---------------------------------------------------------------- host wrapper

_CACHED = {}


def _get_nc():
    if "nc" not in _CACHED:
        _CACHED["nc"] = build_bass()
    return _CACHED["nc"]


def quantize_weight(weight: np.ndarray):
    """Host-side static ternary weight quantization (absmean, {-1,0,+1})."""
    w = np.asarray(weight, dtype=np.float32)
    w_scale = np.float32(np.mean(np.abs(w), dtype=np.float64) + EPS_SCALE)
    w_q = np.clip(np.round(w / w_scale), -1.0, 1.0).astype(ml_dtypes.float8_e4m3)
    return w_q, w_scale


def tile_wq_shard(wqt: np.ndarray, oc_sz: int = 512):
    """[din, o_sh] -> [oc, p, k, c] matching the device DMA layout.

    Row r = 128k + p of wqt lands in partition p at free offset (k, c) of
    oc chunk c // oc_sz.
    """
    din, o_sh = wqt.shape
    k_tiles, oc_chunks = din // P, o_sh // oc_sz
    v = wqt.reshape(k_tiles, P, oc_chunks, oc_sz)
    return np.ascontiguousarray(v.transpose(2, 1, 0, 3))


def kernel(x: np.ndarray, weight: np.ndarray, norm_weight: np.ndarray) -> np.ndarray:
    from concourse.bass_utils import run_bass_kernel_spmd

    assert x.shape == (B, S, DIN) and weight.shape == (DOUT, DIN)
    x_flat = np.ascontiguousarray(x.reshape(TOKENS, DIN), dtype=np.float32)
    w_q, w_scale = quantize_weight(weight)
    gw = np.ascontiguousarray(
        np.broadcast_to(norm_weight.astype(np.float32), (P, DIN)))

    in_maps = []
    for c in range(N_CORES):
        tg, og = divmod(c, OG)
        in_maps.append({
            "x": np.ascontiguousarray(x_flat[tg * T_SH:(tg + 1) * T_SH]),
            "wq": tile_wq_shard(np.ascontiguousarray(
                w_q[og * O_SH:(og + 1) * O_SH, :].T)),
            "gw": gw,
        })

    nc = _get_nc()
    res = run_bass_kernel_spmd(nc, in_maps, core_ids=list(range(N_CORES)))
    _CACHED["last_results"] = res

    out = np.empty((TOKENS, DOUT), dtype=np.float32)
    for c in range(N_CORES):
        tg, og = divmod(c, OG)
        # bf16 -> f32 upcast and the host-side w_scale fold in one pass
        out[tg * T_SH:(tg + 1) * T_SH, og * O_SH:(og + 1) * O_SH] = \
            res.results[c]["out"].astype(np.float32) * w_scale
    return out.reshape(B, S, DOUT)
